# revision 4
# baseline (speedup 1.0000x reference)
"""Paged causal attention (sparse_attention) for 8 Trainium2 NeuronCores.

Strategy: tensor-parallel over heads. Each of the 8 cores gets H/8 = 4 heads,
i.e. a 512-wide column slice of query/key/value/kv_cache/output. block_tables
is read host-side and baked into the DMA gather pattern at build time.

Per-core bass kernel (S=1024 new tokens/seq, P=2048 KV positions/seq, D=128):
  - K/V for each sequence are assembled in SBUF from the paged cache
    (positions < OFF, via block-table runs) and the new key/value tensors
    (positions >= OFF).  The cache update is not an output, so no scatter.
  - scores are computed transposed, tiles [p=128, s=512], with float32r
    matmuls: S_T = K_h^T(chunk) . Q_h^T  (both operands pre-transposed on PE).
  - exp on the scalar engine (scale 1/sqrt(D) fused); causal masking by
    multiplying diagonal tiles with a sliding [128, 896] 0/1 mask.
  - O^T[d, s] accumulates in PSUM via lhsT=V_chunk, rhs=expT_chunk; a
    parallel ones-vector matmul accumulates the softmax denominators.
  - O^T is transposed back on PE and scaled by the reciprocal denominators.
  - fully-masked (future) chunks are skipped in all of QK/exp/AV/denominator.
"""

import sys

if "/opt/trn_rl_repo" not in sys.path:
    sys.path.insert(0, "/opt/trn_rl_repo")

import numpy as np

# Problem constants (hardcoded per the spec; asserted in kernel()).
T, HD = 2048, 4096
NB, BS = 256, 16
B, BLKS = 2, 128
H = 32
NCORES = 8
D = HD // H              # 128
HL = H // NCORES         # 4 heads per core
W = HL * D               # 512 per-core feature width
S = T // B               # 1024 new tokens per sequence
P = BLKS * BS            # 2048 KV positions per sequence
OFF = P - S              # 1024 existing context
NJ = P // 128            # 16 key chunks per sequence
NI = S // 128            # 8 query row-tiles per sequence
SBLK = 512               # s-block width (fp32 moving-operand max)
NK = S // SBLK           # 2 s-blocks per sequence
SCALE = 1.0 / float(np.sqrt(D))

_CACHE = {}


def _cache_runs(bt, b, j):
    """Contiguous-slot runs covering positions [128j, 128j+128) of seq b.

    Returns [(dst_row, src_row, count)] with src_row a row of the flattened
    [NB*BS, :] cache.
    """
    gpos = np.arange(j * 128, j * 128 + 128)
    slots = bt[b, gpos // BS].astype(np.int64) * BS + gpos % BS
    runs = []
    r0 = 0
    for r in range(1, 129):
        if r == 128 or slots[r] != slots[r - 1] + 1:
            runs.append((r0, int(slots[r0]), r - r0))
            r0 = r
    return runs


def _build_nc(bt):
    import concourse.bass as bass
    import concourse.mybir as mybir
    from concourse import bacc
    from concourse.tile import TileContext
    from concourse.masks import make_identity
    from contextlib import ExitStack

    f32 = mybir.dt.float32
    f32r = mybir.dt.float32r
    Exp = mybir.ActivationFunctionType.Exp

    nc = bacc.Bacc("TRN2", target_bir_lowering=False, debug=False,
                   enable_asserts=False)

    q_d = nc.dram_tensor("q", [B * S, W], f32, kind="ExternalInput").ap()
    kn_d = nc.dram_tensor("kn", [B * S, W], f32, kind="ExternalInput").ap()
    vn_d = nc.dram_tensor("vn", [B * S, W], f32r, kind="ExternalInput").ap()
    kc_d = nc.dram_tensor("kc", [NB * BS, W], f32, kind="ExternalInput").ap()
    vc_d = nc.dram_tensor("vc", [NB * BS, W], f32r, kind="ExternalInput").ap()
    o_d = nc.dram_tensor("o", [B * S, W], f32, kind="ExternalOutput").ap()

    with TileContext(nc) as tc, ExitStack() as ctx:
        cpool = ctx.enter_context(tc.tile_pool(name="const", bufs=1))
        kpool = ctx.enter_context(tc.tile_pool(name="k", bufs=2))
        vpool = ctx.enter_context(tc.tile_pool(name="v", bufs=2))
        qpool = ctx.enter_context(tc.tile_pool(name="q", bufs=1))
        ktpool = ctx.enter_context(tc.tile_pool(name="kt", bufs=2))
        qtpool = ctx.enter_context(tc.tile_pool(name="qt", bufs=2))
        expool = ctx.enter_context(tc.tile_pool(name="ex", bufs=4))
        finpool = ctx.enter_context(tc.tile_pool(name="fin", bufs=2))
        outpool = ctx.enter_context(tc.tile_pool(name="outp", bufs=4))
        qkpool = ctx.enter_context(
            tc.tile_pool(name="qk", bufs=2, space="PSUM"))
        otpool = ctx.enter_context(
            tc.tile_pool(name="ot", bufs=2, space="PSUM"))
        rspool = ctx.enter_context(
            tc.tile_pool(name="rs", bufs=2, space="PSUM"))
        trpool = ctx.enter_context(
            tc.tile_pool(name="tr", bufs=2, space="PSUM"))

        ident = cpool.tile([128, 128], f32, name="ident")
        make_identity(nc, ident)
        ones_f = cpool.tile([128, 1], f32, name="ones_f")
        nc.gpsimd.memset(ones_f, 1.0)
        ones = cpool.tile([128, 1], f32r, name="ones")
        nc.vector.tensor_copy(ones, ones_f)
        # bigmask[pi, t] = 1.0 if t - pi >= 384 else 0.0; diagonal tile with
        # base offset `base` uses slice [:, base+384 : base+896].
        bigmask = cpool.tile([128, 896], f32, name="bigmask")
        nc.gpsimd.memset(bigmask, 1.0)
        nc.gpsimd.affine_select(
            out=bigmask, in_=bigmask,
            compare_op=mybir.AluOpType.is_ge,
            fill=0.0, base=-384, channel_multiplier=-1,
            pattern=[[1, 896]],
        )

        for b in range(B):
            # ---- load K/V (cache rows then new rows) and Q for seq b ----
            k_sb = kpool.tile([128, NJ * W], f32, name=f"k_sb{b}", tag="k")
            v_sb = vpool.tile([128, NJ * W], f32r, name=f"v_sb{b}", tag="v")
            for j in range(NJ):
                if j * 128 < OFF:
                    for dst, src, cnt in _cache_runs(bt, b, j):
                        nc.sync.dma_start(
                            k_sb[dst:dst + cnt, j * W:(j + 1) * W],
                            kc_d[src:src + cnt, :])
                        nc.sync.dma_start(
                            v_sb[dst:dst + cnt, j * W:(j + 1) * W],
                            vc_d[src:src + cnt, :])
                else:
                    src = b * S + (j * 128 - OFF)
                    nc.sync.dma_start(
                        k_sb[:, j * W:(j + 1) * W], kn_d[src:src + 128, :])
                    nc.sync.dma_start(
                        v_sb[:, j * W:(j + 1) * W], vn_d[src:src + 128, :])
            q_sb = qpool.tile([128, NI * W], f32, name=f"q_sb{b}", tag="q")
            for i in range(NI):
                src = b * S + i * 128
                nc.sync.dma_start(
                    q_sb[:, i * W:(i + 1) * W], q_d[src:src + 128, :])

            for h in range(HL):
                # ---- build K^T and Q^T for head h ----
                kt_sb = ktpool.tile([128, P], f32r, name=f"kt{b}{h}", tag="kt")
                for j in range(NJ):
                    tr_ps = trpool.tile([128, 128], f32, name="tr_ps",
                                        tag="tr")
                    nc.tensor.transpose(
                        tr_ps, k_sb[:, j * W + h * D:j * W + (h + 1) * D],
                        ident)
                    nc.vector.tensor_copy(
                        kt_sb[:, j * 128:(j + 1) * 128], tr_ps)
                qt_sb = qtpool.tile([128, S], f32r, name=f"qt{b}{h}", tag="qt")
                for i in range(NI):
                    tr_ps = trpool.tile([128, 128], f32, name="tr_ps",
                                        tag="tr")
                    nc.tensor.transpose(
                        tr_ps, q_sb[:, i * W + h * D:i * W + (h + 1) * D],
                        ident)
                    nc.vector.tensor_copy(
                        qt_sb[:, i * 128:(i + 1) * 128], tr_ps)

                for k in range(NK):
                    # chunk j is live unless entirely in the masked future
                    live = [j for j in range(NJ)
                            if OFF + SBLK * k - 128 * j > -SBLK]
                    ot_ps = otpool.tile([128, SBLK], f32, name="ot_ps",
                                        tag="ot")
                    rs_ps = rspool.tile([1, SBLK], f32, name="rs_ps",
                                        tag="rs")
                    for idx, j in enumerate(live):
                        qk_ps = qkpool.tile([128, SBLK], f32, name="qk_ps",
                                            tag="qk")
                        nc.tensor.matmul(
                            qk_ps,
                            lhsT=kt_sb[:, j * 128:(j + 1) * 128],
                            rhs=qt_sb[:, k * SBLK:(k + 1) * SBLK],
                            start=True, stop=True)
                        ex = expool.tile([128, SBLK], f32r, name="ex", tag="ex")
                        nc.scalar.activation(ex, qk_ps, Exp, scale=SCALE)
                        base = OFF + SBLK * k - 128 * j
                        if base <= 126:  # diagonal tile: zero masked entries
                            assert 0 <= base + 384 <= 384 and base % 128 == 0
                            nc.vector.tensor_mul(
                                ex, ex, bigmask[:, base + 384:base + 896])
                        first, last = idx == 0, idx == len(live) - 1
                        nc.tensor.matmul(
                            ot_ps,
                            lhsT=v_sb[:, j * W + h * D:j * W + (h + 1) * D],
                            rhs=ex, start=first, stop=last)
                        nc.tensor.matmul(
                            rs_ps, lhsT=ones[:, 0:1],
                            rhs=ex, start=first, stop=last)

                    # ---- finalize s-block: transpose O^T back, normalize ----
                    rs_sb = finpool.tile([1, SBLK], f32, name="rs_sb",
                                         tag="rs_sb")
                    nc.vector.tensor_copy(rs_sb, rs_ps)
                    ot_sb = finpool.tile([128, SBLK], f32, name="ot_sb",
                                         tag="ot_sb")
                    nc.scalar.copy(ot_sb, ot_ps)
                    for t in range(SBLK // 128):
                        rt_ps = rspool.tile([128, 1], f32, name="rt_ps",
                                            tag="rs")
                        nc.tensor.transpose(
                            rt_ps, rs_sb[0:1, t * 128:(t + 1) * 128],
                            ident[0:1, 0:1])
                        rc_sb = finpool.tile([128, 1], f32, name="rc_sb",
                                             tag="rc")
                        nc.vector.reciprocal(rc_sb, rt_ps)
                        o_ps = trpool.tile([128, 128], f32, name="o_ps",
                                           tag="tr")
                        nc.tensor.transpose(
                            o_ps, ot_sb[:, t * 128:(t + 1) * 128], ident)
                        o_sb = outpool.tile([128, 128], f32, name="o_sb",
                                            tag="o_sb")
                        nc.vector.tensor_scalar_mul(o_sb, o_ps, rc_sb)
                        row = b * S + k * SBLK + t * 128
                        nc.sync.dma_start(
                            o_d[row:row + 128, h * D:(h + 1) * D], o_sb)

    nc.compile()
    return nc


def get_nc(block_tables):
    bt = np.asarray(block_tables)
    key = bt.tobytes()
    if key not in _CACHE:
        _CACHE[key] = _build_nc(bt)
    return _CACHE[key]


def _in_maps(query, key, value, kv_cache):
    maps = []
    for c in range(NCORES):
        cs = slice(c * W, (c + 1) * W)
        maps.append({
            "q": np.ascontiguousarray(query[:, cs]),
            "kn": np.ascontiguousarray(key[:, cs]),
            "vn": np.ascontiguousarray(value[:, cs]),
            "kc": np.ascontiguousarray(
                kv_cache[0].reshape(NB * BS, HD)[:, cs]),
            "vc": np.ascontiguousarray(
                kv_cache[1].reshape(NB * BS, HD)[:, cs]),
        })
    return maps


def run(query, key, value, kv_cache, block_tables, num_heads, **hw_kwargs):
    from concourse import bass_utils

    query = np.asarray(query, dtype=np.float32)
    key = np.asarray(key, dtype=np.float32)
    value = np.asarray(value, dtype=np.float32)
    kv_cache = np.asarray(kv_cache, dtype=np.float32)
    block_tables = np.asarray(block_tables)
    assert int(num_heads) == H
    assert query.shape == (T, HD) and kv_cache.shape == (2, NB, BS, HD)
    assert block_tables.shape == (B, BLKS)

    nc = get_nc(block_tables)
    res = bass_utils.run_bass_kernel_spmd(
        nc, _in_maps(query, key, value, kv_cache),
        core_ids=list(range(NCORES)), **hw_kwargs)
    out = np.concatenate([res.results[c]["o"] for c in range(NCORES)], axis=1)
    return out, res


def kernel(query, key, value, kv_cache, block_tables, num_heads):
    out, _ = run(query, key, value, kv_cache, block_tables, num_heads)
    return out


# revision 5
# speedup vs baseline: 1.1034x; 1.1034x over previous
"""Paged causal attention (sparse_attention) for 8 Trainium2 NeuronCores.

Strategy: tensor-parallel over heads. Each of the 8 cores gets H/8 = 4 heads,
i.e. a 512-wide column slice of query/key/value/kv_cache/output. block_tables
is read host-side and baked into the DMA gather pattern at build time.

Per-core bass kernel (S=1024 new tokens/seq, P=2048 KV positions/seq, D=128):
  - K/V for each sequence are assembled in SBUF from the paged cache
    (positions < OFF, via block-table runs) and the new key/value tensors
    (positions >= OFF); GPSIMD casts everything to bf16 (matmuls run at
    1 cycle/row in bf16 vs 2 for fp32r).  The cache update is not an
    output, so no scatter is needed.
  - scores are computed transposed, tiles [p=128, s=512]:
    S_T = K_h^T(chunk) . Q_h^T, with K^T/Q^T built by PE transposes that are
    interleaved into the previous head's matmul stream (keeps HAM warm).
  - exp on the scalar engine (scale 1/sqrt(D) fused), bf16 out; causal
    masking multiplies diagonal tiles with a sliding [128, 896] 0/1 mask on
    GPSIMD (DVE stays light).
  - O^T[d, s] accumulates in PSUM via lhsT=V_chunk, rhs=expT_chunk; a
    parallel ones-vector matmul accumulates the softmax denominators.
  - O^T is transposed back on PE and scaled by the reciprocal denominators.
  - fully-masked (future) chunks are skipped in all of QK/exp/AV/denominator.
"""

import sys

if "/opt/trn_rl_repo" not in sys.path:
    sys.path.insert(0, "/opt/trn_rl_repo")

import numpy as np

# Problem constants (hardcoded per the spec; asserted in kernel()).
T, HD = 2048, 4096
NB, BS = 256, 16
B, BLKS = 2, 128
H = 32
NCORES = 8
D = HD // H              # 128
HL = H // NCORES         # 4 heads per core
W = HL * D               # 512 per-core feature width
S = T // B               # 1024 new tokens per sequence
P = BLKS * BS            # 2048 KV positions per sequence
OFF = P - S              # 1024 existing context
NJ = P // 128            # 16 key chunks per sequence
NI = S // 128            # 8 query row-tiles per sequence
SBLK = 512               # s-block width (one PSUM bank of fp32)
NK = S // SBLK           # 2 s-blocks per sequence
SCALE = 1.0 / float(np.sqrt(D))

_CACHE = {}


def _cache_runs(bt, b, j):
    """Contiguous-slot runs covering positions [128j, 128j+128) of seq b.

    Returns [(dst_row, src_row, count)] with src_row a row of the flattened
    [NB*BS, :] cache.
    """
    gpos = np.arange(j * 128, j * 128 + 128)
    slots = bt[b, gpos // BS].astype(np.int64) * BS + gpos % BS
    runs = []
    r0 = 0
    for r in range(1, 129):
        if r == 128 or slots[r] != slots[r - 1] + 1:
            runs.append((r0, int(slots[r0]), r - r0))
            r0 = r
    return runs


def _build_nc(bt):
    import concourse.bass as bass
    import concourse.mybir as mybir
    from concourse import bacc
    from concourse.tile import TileContext
    from concourse.masks import make_identity
    from contextlib import ExitStack

    f32 = mybir.dt.float32
    bf16 = mybir.dt.bfloat16
    Exp = mybir.ActivationFunctionType.Exp

    nc = bacc.Bacc("TRN2", target_bir_lowering=False, debug=False,
                   enable_asserts=False)

    q_d = nc.dram_tensor("q", [B * S, W], f32, kind="ExternalInput").ap()
    kn_d = nc.dram_tensor("kn", [B * S, W], f32, kind="ExternalInput").ap()
    vn_d = nc.dram_tensor("vn", [B * S, W], f32, kind="ExternalInput").ap()
    kc_d = nc.dram_tensor("kc", [NB * BS, W], f32, kind="ExternalInput").ap()
    vc_d = nc.dram_tensor("vc", [NB * BS, W], f32, kind="ExternalInput").ap()
    o_d = nc.dram_tensor("o", [B * S, W], f32, kind="ExternalOutput").ap()

    with TileContext(nc) as tc, ExitStack() as ctx:
        cpool = ctx.enter_context(tc.tile_pool(name="const", bufs=1))
        stpool = ctx.enter_context(tc.tile_pool(name="stage", bufs=6))
        kpool = ctx.enter_context(tc.tile_pool(name="kbf", bufs=2))
        vpool = ctx.enter_context(tc.tile_pool(name="vbf", bufs=2))
        qpool = ctx.enter_context(tc.tile_pool(name="qbf", bufs=2))
        ktpool = ctx.enter_context(tc.tile_pool(name="kt", bufs=2))
        qtpool = ctx.enter_context(tc.tile_pool(name="qt", bufs=2))
        expool = ctx.enter_context(tc.tile_pool(name="ex", bufs=6))
        finpool = ctx.enter_context(tc.tile_pool(name="fin", bufs=2))
        outpool = ctx.enter_context(tc.tile_pool(name="outp", bufs=4))
        qkpool = ctx.enter_context(
            tc.tile_pool(name="qk", bufs=2, space="PSUM"))
        otpool = ctx.enter_context(
            tc.tile_pool(name="ot", bufs=2, space="PSUM"))
        rspool = ctx.enter_context(
            tc.tile_pool(name="rs", bufs=2, space="PSUM"))
        trpool = ctx.enter_context(
            tc.tile_pool(name="tr", bufs=2, space="PSUM"))

        ident = cpool.tile([128, 128], f32, name="ident")
        make_identity(nc, ident)
        ident_bf = cpool.tile([128, 128], bf16, name="ident_bf")
        nc.vector.tensor_copy(ident_bf, ident)
        ones_f = cpool.tile([128, 1], f32, name="ones_f")
        nc.gpsimd.memset(ones_f, 1.0)
        ones = cpool.tile([128, 1], bf16, name="ones")
        nc.vector.tensor_copy(ones, ones_f)
        # bigmask[pi, t] = 1.0 if t - pi >= 384 else 0.0; diagonal tile with
        # base offset `base` uses slice [:, base+384 : base+896].
        bigmask_f = cpool.tile([128, 896], f32, name="bigmask_f")
        nc.gpsimd.memset(bigmask_f, 1.0)
        nc.gpsimd.affine_select(
            out=bigmask_f, in_=bigmask_f,
            compare_op=mybir.AluOpType.is_ge,
            fill=0.0, base=-384, channel_multiplier=-1,
            pattern=[[1, 896]],
        )
        bigmask = cpool.tile([128, 896], bf16, name="bigmask")
        nc.vector.tensor_copy(bigmask, bigmask_f)

        def load_cast(dst_bf, j, runs):
            """DMA one 128-row chunk into a stage tile, cast bf16 into dst."""
            st = stpool.tile([128, W], f32, name="st", tag="st")
            for dst, (dram, src, cnt) in runs:
                nc.sync.dma_start(st[dst:dst + cnt, :],
                                  dram[src:src + cnt, :])
            nc.gpsimd.tensor_copy(dst_bf[:, j * W:(j + 1) * W], st)

        def kv_runs(b, j, new_d, cache_d):
            if j * 128 < OFF:
                return [(dst, (cache_d, src, cnt))
                        for dst, src, cnt in _cache_runs(bt, b, j)]
            return [(0, (new_d, b * S + (j * 128 - OFF), 128))]

        # Per (b, h) transpose work is emitted lazily so it can be
        # interleaved into the previous head's matmul stream (keeps the PE
        # HAM clock-gate warm: transpose-mode doesn't count as PE-busy).
        def make_transpose_ops(k_bf, q_bf, h, tag):
            kt_sb = ktpool.tile([128, P], bf16, name=f"kt{tag}", tag="kt")
            qt_sb = qtpool.tile([128, S], bf16, name=f"qt{tag}", tag="qt")
            ops = []

            def tr(src_sb, col0, dst_sb, dcol0):
                def run():
                    tr_ps = trpool.tile([128, 128], bf16, name="tr_ps",
                                        tag="tr")
                    nc.tensor.transpose(
                        tr_ps, src_sb[:, col0:col0 + 128], ident_bf)
                    nc.vector.tensor_copy(
                        dst_sb[:, dcol0:dcol0 + 128], tr_ps)
                return run

            for j in range(NJ):
                ops.append(tr(k_bf, j * W + h * D, kt_sb, j * 128))
            for i in range(NI):
                ops.append(tr(q_bf, i * W + h * D, qt_sb, i * 128))
            return kt_sb, qt_sb, ops

        # Stage 1: load + cast all sequences' K/V/Q up front (DMA + GPSIMD
        # run ahead; SBUF holds both seqs in bf16: (16+16+8)KB * 2).
        kq = []  # per b: (k_bf, v_bf, q_bf)
        for b in range(B):
            k_bf = kpool.tile([128, NJ * W], bf16, name=f"k_bf{b}", tag="k")
            v_bf = vpool.tile([128, NJ * W], bf16, name=f"v_bf{b}", tag="v")
            q_bf = qpool.tile([128, NI * W], bf16, name=f"q_bf{b}", tag="q")
            for j in range(NJ):
                load_cast(k_bf, j, kv_runs(b, j, kn_d, kc_d))
                load_cast(v_bf, j, kv_runs(b, j, vn_d, vc_d))
            for i in range(NI):
                load_cast(q_bf, i, [(0, (q_d, b * S + i * 128, 128))])
            kq.append((k_bf, v_bf, q_bf))

        # Stage 2: per (b, h): matmul stream with next head's transposes
        # interleaved.
        heads = [(b, h) for b in range(B) for h in range(HL)]
        k_bf, v_bf, q_bf = kq[0]
        kt_sb, qt_sb, ops0 = make_transpose_ops(k_bf, q_bf, 0, "00")
        for op in ops0:
            op()
        pending = []  # transpose ops for the next head, emitted interleaved

        for hi, (b, h) in enumerate(heads):
            k_bf, v_bf, q_bf = kq[b]
            if hi + 1 < len(heads):
                nb_, nh = heads[hi + 1]
                nkt, nqt, pending = make_transpose_ops(
                    kq[nb_][0], kq[nb_][2], nh, f"{nb_}{nh}")
            else:
                nkt, nqt, pending = None, None, []

            for k in range(NK):
                live = [j for j in range(NJ)
                        if OFF + SBLK * k - 128 * j > -SBLK]
                ot_ps = otpool.tile([128, SBLK], f32, name="ot_ps", tag="ot")
                rs_ps = rspool.tile([1, SBLK], f32, name="rs_ps", tag="rs")
                for idx, j in enumerate(live):
                    qk_ps = qkpool.tile([128, SBLK], f32, name="qk_ps",
                                        tag="qk")
                    nc.tensor.matmul(
                        qk_ps,
                        lhsT=kt_sb[:, j * 128:(j + 1) * 128],
                        rhs=qt_sb[:, k * SBLK:(k + 1) * SBLK],
                        start=True, stop=True)
                    ex = expool.tile([128, SBLK], bf16, name="ex", tag="ex")
                    nc.scalar.activation(ex, qk_ps, Exp, scale=SCALE)
                    base = OFF + SBLK * k - 128 * j
                    if base <= 126:  # diagonal tile: zero masked entries
                        assert 0 <= base + 384 <= 384 and base % 128 == 0
                        nc.gpsimd.tensor_mul(
                            ex, ex, bigmask[:, base + 384:base + 896])
                    first, last = idx == 0, idx == len(live) - 1
                    nc.tensor.matmul(
                        ot_ps,
                        lhsT=v_bf[:, j * W + h * D:j * W + (h + 1) * D],
                        rhs=ex, start=first, stop=last)
                    nc.tensor.matmul(
                        rs_ps, lhsT=ones[:, 0:1],
                        rhs=ex, start=first, stop=last)
                    # drip next head's transposes into the PE stream
                    if pending:
                        pending.pop(0)()
                    if pending and idx % 2 == 0:
                        pending.pop(0)()

                # ---- finalize s-block: transpose O^T back, normalize ----
                rs_sb = finpool.tile([1, SBLK], f32, name="rs_sb",
                                     tag="rs_sb")
                nc.vector.tensor_copy(rs_sb, rs_ps)
                ot_sb = finpool.tile([128, SBLK], bf16, name="ot_sb",
                                     tag="ot_sb")
                nc.scalar.copy(ot_sb, ot_ps)
                for t in range(SBLK // 128):
                    rt_ps = rspool.tile([128, 1], f32, name="rt_ps",
                                        tag="rs")
                    nc.tensor.transpose(
                        rt_ps, rs_sb[0:1, t * 128:(t + 1) * 128],
                        ident[0:1, 0:1])
                    rc_sb = finpool.tile([128, 1], f32, name="rc_sb",
                                         tag="rc")
                    nc.vector.reciprocal(rc_sb, rt_ps)
                    o_ps = trpool.tile([128, 128], bf16, name="o_ps",
                                       tag="tr")
                    nc.tensor.transpose(
                        o_ps, ot_sb[:, t * 128:(t + 1) * 128], ident_bf)
                    o_sb = outpool.tile([128, 128], f32, name="o_sb",
                                        tag="o_sb")
                    nc.vector.tensor_scalar_mul(o_sb, o_ps, rc_sb)
                    row = b * S + k * SBLK + t * 128
                    nc.sync.dma_start(
                        o_d[row:row + 128, h * D:(h + 1) * D], o_sb)

            # any leftover transposes for the next head
            for op in pending:
                op()
            if nkt is not None:
                kt_sb, qt_sb = nkt, nqt

    nc.compile()
    return nc


def get_nc(block_tables):
    bt = np.asarray(block_tables)
    key = bt.tobytes()
    if key not in _CACHE:
        _CACHE[key] = _build_nc(bt)
    return _CACHE[key]


def _in_maps(query, key, value, kv_cache):
    maps = []
    for c in range(NCORES):
        cs = slice(c * W, (c + 1) * W)
        maps.append({
            "q": np.ascontiguousarray(query[:, cs]),
            "kn": np.ascontiguousarray(key[:, cs]),
            "vn": np.ascontiguousarray(value[:, cs]),
            "kc": np.ascontiguousarray(
                kv_cache[0].reshape(NB * BS, HD)[:, cs]),
            "vc": np.ascontiguousarray(
                kv_cache[1].reshape(NB * BS, HD)[:, cs]),
        })
    return maps


def run(query, key, value, kv_cache, block_tables, num_heads, **hw_kwargs):
    from concourse import bass_utils

    query = np.asarray(query, dtype=np.float32)
    key = np.asarray(key, dtype=np.float32)
    value = np.asarray(value, dtype=np.float32)
    kv_cache = np.asarray(kv_cache, dtype=np.float32)
    block_tables = np.asarray(block_tables)
    assert int(num_heads) == H
    assert query.shape == (T, HD) and kv_cache.shape == (2, NB, BS, HD)
    assert block_tables.shape == (B, BLKS)

    nc = get_nc(block_tables)
    res = bass_utils.run_bass_kernel_spmd(
        nc, _in_maps(query, key, value, kv_cache),
        core_ids=list(range(NCORES)), **hw_kwargs)
    out = np.concatenate([res.results[c]["o"] for c in range(NCORES)], axis=1)
    return out, res


def kernel(query, key, value, kv_cache, block_tables, num_heads):
    out, _ = run(query, key, value, kv_cache, block_tables, num_heads)
    return out


# revision 6
# speedup vs baseline: 1.4259x; 1.2923x over previous
"""Paged causal attention (sparse_attention) for 8 Trainium2 NeuronCores.

Strategy: tensor-parallel over heads. Each of the 8 cores gets H/8 = 4 heads,
i.e. a 512-wide column slice of query/key/value/kv_cache/output. block_tables
is read host-side and baked into the DMA gather pattern at build time.

Per-core bass kernel (S=1024 new tokens/seq, P=2048 KV positions/seq, D=128):
  - K/V for each sequence are assembled in SBUF from the paged cache
    (positions < OFF, via block-table runs) and the new key/value tensors
    (positions >= OFF); GPSIMD casts everything to bf16 (matmuls run at
    1 cycle/row in bf16 vs 2 for fp32r).  The cache update is not an
    output, so no scatter is needed.
  - scores are computed transposed, tiles [p=128, s=512]:
    S_T = K_h^T(chunk) . Q_h^T, with K^T/Q^T built by PE transposes that are
    interleaved into the previous head's matmul stream (keeps HAM warm).
  - exp on the scalar engine (scale 1/sqrt(D) fused), bf16 out; causal
    masking multiplies diagonal tiles with a sliding [128, 896] 0/1 mask on
    GPSIMD (DVE stays light).
  - O^T[d, s] accumulates in PSUM via lhsT=V_chunk, rhs=expT_chunk; a
    parallel ones-vector matmul accumulates the softmax denominators.
  - O^T is transposed back on PE and scaled by the reciprocal denominators.
  - fully-masked (future) chunks are skipped in all of QK/exp/AV/denominator.
"""

import sys

if "/opt/trn_rl_repo" not in sys.path:
    sys.path.insert(0, "/opt/trn_rl_repo")

import numpy as np

# Problem constants (hardcoded per the spec; asserted in kernel()).
T, HD = 2048, 4096
NB, BS = 256, 16
B, BLKS = 2, 128
H = 32
NCORES = 8
D = HD // H              # 128
HL = H // NCORES         # 4 heads per core
W = HL * D               # 512 per-core feature width
S = T // B               # 1024 new tokens per sequence
P = BLKS * BS            # 2048 KV positions per sequence
OFF = P - S              # 1024 existing context
NJ = P // 128            # 16 key chunks per sequence
NI = S // 128            # 8 query row-tiles per sequence
SBLK = 512               # s-block width (one PSUM bank of fp32)
NK = S // SBLK           # 2 s-blocks per sequence
SCALE = 1.0 / float(np.sqrt(D))

_CACHE = {}


def _cache_runs(bt, b, j):
    """Contiguous-slot runs covering positions [128j, 128j+128) of seq b.

    Returns [(dst_row, src_row, count)] with src_row a row of the flattened
    [NB*BS, :] cache.
    """
    gpos = np.arange(j * 128, j * 128 + 128)
    slots = bt[b, gpos // BS].astype(np.int64) * BS + gpos % BS
    runs = []
    r0 = 0
    for r in range(1, 129):
        if r == 128 or slots[r] != slots[r - 1] + 1:
            runs.append((r0, int(slots[r0]), r - r0))
            r0 = r
    return runs


def _build_nc(bt):
    import concourse.bass as bass
    import concourse.mybir as mybir
    from concourse import bacc
    from concourse.tile import TileContext
    from concourse.masks import make_identity
    from contextlib import ExitStack

    f32 = mybir.dt.float32
    bf16 = mybir.dt.bfloat16
    Exp = mybir.ActivationFunctionType.Exp

    nc = bacc.Bacc("TRN2", target_bir_lowering=False, debug=False,
                   enable_asserts=False)

    q_d = nc.dram_tensor("q", [B * S, W], f32, kind="ExternalInput").ap()
    kn_d = nc.dram_tensor("kn", [B * S, W], f32, kind="ExternalInput").ap()
    vn_d = nc.dram_tensor("vn", [B * S, W], f32, kind="ExternalInput").ap()
    kc_d = nc.dram_tensor("kc", [NB * BS, W], f32, kind="ExternalInput").ap()
    vc_d = nc.dram_tensor("vc", [NB * BS, W], f32, kind="ExternalInput").ap()
    o_d = nc.dram_tensor("o", [B * S, W], f32, kind="ExternalOutput").ap()

    with TileContext(nc) as tc, ExitStack() as ctx:
        cpool = ctx.enter_context(tc.tile_pool(name="const", bufs=1))
        stpool = ctx.enter_context(tc.tile_pool(name="stage", bufs=6))
        kpool = ctx.enter_context(tc.tile_pool(name="kbf", bufs=2))
        vpool = ctx.enter_context(tc.tile_pool(name="vbf", bufs=2))
        qpool = ctx.enter_context(tc.tile_pool(name="qbf", bufs=2))
        ktpool = ctx.enter_context(tc.tile_pool(name="kt", bufs=2))
        qtpool = ctx.enter_context(tc.tile_pool(name="qt", bufs=2))
        expool = ctx.enter_context(tc.tile_pool(name="ex", bufs=6))
        finpool = ctx.enter_context(tc.tile_pool(name="fin", bufs=2))
        outpool = ctx.enter_context(tc.tile_pool(name="outp", bufs=4))
        qkpool = ctx.enter_context(
            tc.tile_pool(name="qk", bufs=3, space="PSUM"))
        otpool = ctx.enter_context(
            tc.tile_pool(name="ot", bufs=2, space="PSUM"))
        rspool = ctx.enter_context(
            tc.tile_pool(name="rs", bufs=1, space="PSUM"))
        trpool = ctx.enter_context(
            tc.tile_pool(name="tr", bufs=2, space="PSUM"))

        ident = cpool.tile([128, 128], f32, name="ident")
        make_identity(nc, ident)
        ident_bf = cpool.tile([128, 128], bf16, name="ident_bf")
        nc.vector.tensor_copy(ident_bf, ident)
        ones_f = cpool.tile([128, 1], f32, name="ones_f")
        nc.gpsimd.memset(ones_f, 1.0)
        ones = cpool.tile([128, 1], bf16, name="ones")
        nc.vector.tensor_copy(ones, ones_f)
        # bigmask[pi, t] = 1.0 if t - pi >= 384 else 0.0; diagonal tile with
        # base offset `base` uses slice [:, base+384 : base+896].
        bigmask_f = cpool.tile([128, 896], f32, name="bigmask_f")
        nc.gpsimd.memset(bigmask_f, 1.0)
        nc.gpsimd.affine_select(
            out=bigmask_f, in_=bigmask_f,
            compare_op=mybir.AluOpType.is_ge,
            fill=0.0, base=-384, channel_multiplier=-1,
            pattern=[[1, 896]],
        )
        bigmask = cpool.tile([128, 896], bf16, name="bigmask")
        nc.vector.tensor_copy(bigmask, bigmask_f)

        def load_cast(dst_bf, j, runs):
            """DMA one 128-row chunk into a stage tile, cast bf16 into dst."""
            st = stpool.tile([128, W], f32, name="st", tag="st")
            for dst, (dram, src, cnt) in runs:
                nc.sync.dma_start(st[dst:dst + cnt, :],
                                  dram[src:src + cnt, :])
            nc.vector.tensor_copy(dst_bf[:, j * W:(j + 1) * W], st)

        def kv_runs(b, j, new_d, cache_d):
            if j * 128 < OFF:
                return [(dst, (cache_d, src, cnt))
                        for dst, src, cnt in _cache_runs(bt, b, j)]
            return [(0, (new_d, b * S + (j * 128 - OFF), 128))]

        # Per (b, h) transpose work is emitted lazily so it can be
        # interleaved into the previous head's matmul stream (keeps the PE
        # HAM clock-gate warm: transpose-mode doesn't count as PE-busy).
        def make_transpose_ops(k_bf, q_bf, h, tag):
            kt_sb = ktpool.tile([128, P], bf16, name=f"kt{tag}", tag="kt")
            qt_sb = qtpool.tile([128, S], bf16, name=f"qt{tag}", tag="qt")
            ops = []

            def tr(src_sb, col0, dst_sb, dcol0):
                def run():
                    tr_ps = trpool.tile([128, 128], bf16, name="tr_ps",
                                        tag="tr")
                    nc.tensor.transpose(
                        tr_ps, src_sb[:, col0:col0 + 128], ident_bf)
                    nc.vector.tensor_copy(
                        dst_sb[:, dcol0:dcol0 + 128], tr_ps)
                return run

            for j in range(NJ):
                ops.append(tr(k_bf, j * W + h * D, kt_sb, j * 128))
            for i in range(NI):
                ops.append(tr(q_bf, i * W + h * D, qt_sb, i * 128))
            return kt_sb, qt_sb, ops

        # Stage 1: load + cast all sequences' K/V/Q up front (DMA + GPSIMD
        # run ahead; SBUF holds both seqs in bf16: (16+16+8)KB * 2).
        kq = []  # per b: (k_bf, v_bf, q_bf)
        for b in range(B):
            k_bf = kpool.tile([128, NJ * W], bf16, name=f"k_bf{b}", tag="k")
            v_bf = vpool.tile([128, NJ * W], bf16, name=f"v_bf{b}", tag="v")
            q_bf = qpool.tile([128, NI * W], bf16, name=f"q_bf{b}", tag="q")
            for j in range(NJ):
                load_cast(k_bf, j, kv_runs(b, j, kn_d, kc_d))
                load_cast(v_bf, j, kv_runs(b, j, vn_d, vc_d))
            for i in range(NI):
                load_cast(q_bf, i, [(0, (q_d, b * S + i * 128, 128))])
            kq.append((k_bf, v_bf, q_bf))

        # Stage 2: per (b, h): matmul stream with next head's transposes
        # interleaved.
        heads = [(b, h) for b in range(B) for h in range(HL)]
        k_bf, v_bf, q_bf = kq[0]
        kt_sb, qt_sb, ops0 = make_transpose_ops(k_bf, q_bf, 0, "00")
        for op in ops0:
            op()
        pending = []  # transpose ops for the next head, emitted interleaved

        for hi, (b, h) in enumerate(heads):
            k_bf, v_bf, q_bf = kq[b]
            if hi + 1 < len(heads):
                nb_, nh = heads[hi + 1]
                nkt, nqt, pending = make_transpose_ops(
                    kq[nb_][0], kq[nb_][2], nh, f"{nb_}{nh}")
            else:
                nkt, nqt, pending = None, None, []

            for k in range(NK):
                live = [j for j in range(NJ)
                        if OFF + SBLK * k - 128 * j > -SBLK]
                ot_ps = otpool.tile([128, SBLK], f32, name="ot_ps", tag="ot")
                rs_ps = rspool.tile([1, SBLK], f32, name="rs_ps", tag="rs")
                def emit_av(idx, j, ex):
                    first, last = idx == 0, idx == len(live) - 1
                    nc.tensor.matmul(
                        ot_ps,
                        lhsT=v_bf[:, j * W + h * D:j * W + (h + 1) * D],
                        rhs=ex, start=first, stop=last)
                    nc.tensor.matmul(
                        rs_ps, lhsT=ones[:, 0:1],
                        rhs=ex, start=first, stop=last)

                prev = None  # (idx, j, ex) whose AV/RS is not yet emitted
                for idx, j in enumerate(live):
                    qk_ps = qkpool.tile([128, SBLK], f32, name="qk_ps",
                                        tag="qk")
                    nc.tensor.matmul(
                        qk_ps,
                        lhsT=kt_sb[:, j * 128:(j + 1) * 128],
                        rhs=qt_sb[:, k * SBLK:(k + 1) * SBLK],
                        start=True, stop=True)
                    ex = expool.tile([128, SBLK], bf16, name="ex", tag="ex")
                    nc.scalar.activation(ex, qk_ps, Exp, scale=SCALE)
                    base = OFF + SBLK * k - 128 * j
                    if base <= 126:  # diagonal tile: zero masked entries
                        assert 0 <= base + 384 <= 384 and base % 128 == 0
                        nc.gpsimd.tensor_mul(
                            ex, ex, bigmask[:, base + 384:base + 896])
                    if prev is not None:
                        emit_av(*prev)
                    prev = (idx, j, ex)
                    # drip next head's transposes into the PE stream
                    if pending:
                        pending.pop(0)()
                    if pending and idx % 2 == 0:
                        pending.pop(0)()
                emit_av(*prev)

                # ---- finalize s-block: transpose O^T back, normalize ----
                rs_sb = finpool.tile([1, SBLK], f32, name="rs_sb",
                                     tag="rs_sb")
                nc.vector.tensor_copy(rs_sb, rs_ps)
                ot_sb = finpool.tile([128, SBLK], bf16, name="ot_sb",
                                     tag="ot_sb")
                nc.scalar.copy(ot_sb, ot_ps)
                for t in range(SBLK // 128):
                    rt_ps = rspool.tile([128, 1], f32, name="rt_ps",
                                        tag="rs")
                    nc.tensor.transpose(
                        rt_ps, rs_sb[0:1, t * 128:(t + 1) * 128],
                        ident[0:1, 0:1])
                    rc_sb = finpool.tile([128, 1], f32, name="rc_sb",
                                         tag="rc")
                    nc.vector.reciprocal(rc_sb, rt_ps)
                    o_ps = trpool.tile([128, 128], bf16, name="o_ps",
                                       tag="tr")
                    nc.tensor.transpose(
                        o_ps, ot_sb[:, t * 128:(t + 1) * 128], ident_bf)
                    o_sb = outpool.tile([128, 128], f32, name="o_sb",
                                        tag="o_sb")
                    nc.vector.tensor_scalar_mul(o_sb, o_ps, rc_sb)
                    row = b * S + k * SBLK + t * 128
                    nc.sync.dma_start(
                        o_d[row:row + 128, h * D:(h + 1) * D], o_sb)

            # any leftover transposes for the next head
            for op in pending:
                op()
            if nkt is not None:
                kt_sb, qt_sb = nkt, nqt

    nc.compile()
    return nc


def get_nc(block_tables):
    bt = np.asarray(block_tables)
    key = bt.tobytes()
    if key not in _CACHE:
        _CACHE[key] = _build_nc(bt)
    return _CACHE[key]


def _in_maps(query, key, value, kv_cache):
    maps = []
    for c in range(NCORES):
        cs = slice(c * W, (c + 1) * W)
        maps.append({
            "q": np.ascontiguousarray(query[:, cs]),
            "kn": np.ascontiguousarray(key[:, cs]),
            "vn": np.ascontiguousarray(value[:, cs]),
            "kc": np.ascontiguousarray(
                kv_cache[0].reshape(NB * BS, HD)[:, cs]),
            "vc": np.ascontiguousarray(
                kv_cache[1].reshape(NB * BS, HD)[:, cs]),
        })
    return maps


def run(query, key, value, kv_cache, block_tables, num_heads, **hw_kwargs):
    from concourse import bass_utils

    query = np.asarray(query, dtype=np.float32)
    key = np.asarray(key, dtype=np.float32)
    value = np.asarray(value, dtype=np.float32)
    kv_cache = np.asarray(kv_cache, dtype=np.float32)
    block_tables = np.asarray(block_tables)
    assert int(num_heads) == H
    assert query.shape == (T, HD) and kv_cache.shape == (2, NB, BS, HD)
    assert block_tables.shape == (B, BLKS)

    nc = get_nc(block_tables)
    res = bass_utils.run_bass_kernel_spmd(
        nc, _in_maps(query, key, value, kv_cache),
        core_ids=list(range(NCORES)), **hw_kwargs)
    out = np.concatenate([res.results[c]["o"] for c in range(NCORES)], axis=1)
    return out, res


def kernel(query, key, value, kv_cache, block_tables, num_heads):
    out, _ = run(query, key, value, kv_cache, block_tables, num_heads)
    return out


# revision 8
# speedup vs baseline: 1.4353x; 1.0066x over previous
"""Paged causal attention (sparse_attention) for 8 Trainium2 NeuronCores.

Strategy: tensor-parallel over heads. Each of the 8 cores gets H/8 = 4 heads,
i.e. a 512-wide column slice of query/key/value/kv_cache/output. block_tables
is read host-side and baked into the DMA gather pattern at build time.

Per-core bass kernel (S=1024 new tokens/seq, P=2048 KV positions/seq, D=128):
  - K/V for each sequence are assembled in SBUF from the paged cache
    (positions < OFF, via block-table runs) and the new key/value tensors
    (positions >= OFF); GPSIMD casts everything to bf16 (matmuls run at
    1 cycle/row in bf16 vs 2 for fp32r).  The cache update is not an
    output, so no scatter is needed.
  - scores are computed transposed, tiles [p=128, s=512]:
    S_T = K_h^T(chunk) . Q_h^T, with K^T/Q^T built by PE transposes that are
    interleaved into the previous head's matmul stream (keeps HAM warm).
  - exp on the scalar engine (scale 1/sqrt(D) fused), bf16 out; causal
    masking multiplies diagonal tiles with a sliding [128, 896] 0/1 mask on
    GPSIMD (DVE stays light).
  - O^T[d, s] accumulates in PSUM via lhsT=V_chunk, rhs=expT_chunk; a
    parallel ones-vector matmul accumulates the softmax denominators.
  - O^T is transposed back on PE and scaled by the reciprocal denominators.
  - fully-masked (future) chunks are skipped in all of QK/exp/AV/denominator.
"""

import sys

if "/opt/trn_rl_repo" not in sys.path:
    sys.path.insert(0, "/opt/trn_rl_repo")

import numpy as np

# Problem constants (hardcoded per the spec; asserted in kernel()).
T, HD = 2048, 4096
NB, BS = 256, 16
B, BLKS = 2, 128
H = 32
NCORES = 8
D = HD // H              # 128
HL = H // NCORES         # 4 heads per core
W = HL * D               # 512 per-core feature width
S = T // B               # 1024 new tokens per sequence
P = BLKS * BS            # 2048 KV positions per sequence
OFF = P - S              # 1024 existing context
NJ = P // 128            # 16 key chunks per sequence
NI = S // 128            # 8 query row-tiles per sequence
SBLK = 512               # s-block width (one PSUM bank of fp32)
NK = S // SBLK           # 2 s-blocks per sequence
SCALE = 1.0 / float(np.sqrt(D))

_CACHE = {}


def _cache_runs(bt, b, j):
    """Contiguous-slot runs covering positions [128j, 128j+128) of seq b.

    Returns [(dst_row, src_row, count)] with src_row a row of the flattened
    [NB*BS, :] cache.
    """
    gpos = np.arange(j * 128, j * 128 + 128)
    slots = bt[b, gpos // BS].astype(np.int64) * BS + gpos % BS
    runs = []
    r0 = 0
    for r in range(1, 129):
        if r == 128 or slots[r] != slots[r - 1] + 1:
            runs.append((r0, int(slots[r0]), r - r0))
            r0 = r
    return runs


def _build_nc(bt):
    import concourse.bass as bass
    import concourse.mybir as mybir
    from concourse import bacc
    from concourse.tile import TileContext
    from concourse.masks import make_identity
    from contextlib import ExitStack

    f32 = mybir.dt.float32
    bf16 = mybir.dt.bfloat16
    Exp = mybir.ActivationFunctionType.Exp

    nc = bacc.Bacc("TRN2", target_bir_lowering=False, debug=False,
                   enable_asserts=False)

    q_d = nc.dram_tensor("q", [B * S, W], f32, kind="ExternalInput").ap()
    kn_d = nc.dram_tensor("kn", [B * S, W], f32, kind="ExternalInput").ap()
    vn_d = nc.dram_tensor("vn", [B * S, W], f32, kind="ExternalInput").ap()
    kc_d = nc.dram_tensor("kc", [NB * BS, W], f32, kind="ExternalInput").ap()
    vc_d = nc.dram_tensor("vc", [NB * BS, W], f32, kind="ExternalInput").ap()
    o_d = nc.dram_tensor("o", [B * S, W], f32, kind="ExternalOutput").ap()

    with TileContext(nc) as tc, ExitStack() as ctx:
        cpool = ctx.enter_context(tc.tile_pool(name="const", bufs=1))
        stpool = ctx.enter_context(tc.tile_pool(name="stage", bufs=6))
        kpool = ctx.enter_context(tc.tile_pool(name="kbf", bufs=2))
        vpool = ctx.enter_context(tc.tile_pool(name="vbf", bufs=2))
        qpool = ctx.enter_context(tc.tile_pool(name="qbf", bufs=2))
        ktpool = ctx.enter_context(tc.tile_pool(name="kt", bufs=2))
        qtpool = ctx.enter_context(tc.tile_pool(name="qt", bufs=2))
        expool = ctx.enter_context(tc.tile_pool(name="ex", bufs=18))
        finpool = ctx.enter_context(tc.tile_pool(name="fin", bufs=2))
        outpool = ctx.enter_context(tc.tile_pool(name="outp", bufs=4))
        qkpool = ctx.enter_context(
            tc.tile_pool(name="qk", bufs=2, space="PSUM"))
        otpool = ctx.enter_context(
            tc.tile_pool(name="ot", bufs=2, space="PSUM"))
        rspool = ctx.enter_context(
            tc.tile_pool(name="rs", bufs=1, space="PSUM"))
        trpool = ctx.enter_context(
            tc.tile_pool(name="tr", bufs=1, space="PSUM"))

        ident = cpool.tile([128, 128], f32, name="ident")
        make_identity(nc, ident)
        ident_bf = cpool.tile([128, 128], bf16, name="ident_bf")
        nc.vector.tensor_copy(ident_bf, ident)
        ones_f = cpool.tile([128, 1], f32, name="ones_f")
        nc.gpsimd.memset(ones_f, 1.0)
        ones = cpool.tile([128, 1], bf16, name="ones")
        nc.vector.tensor_copy(ones, ones_f)
        # bigmask[pi, t] = 1.0 if t - pi >= 384 else 0.0; diagonal tile with
        # base offset `base` uses slice [:, base+384 : base+896].
        bigmask_f = cpool.tile([128, 896], f32, name="bigmask_f")
        nc.gpsimd.memset(bigmask_f, 1.0)
        nc.gpsimd.affine_select(
            out=bigmask_f, in_=bigmask_f,
            compare_op=mybir.AluOpType.is_ge,
            fill=0.0, base=-384, channel_multiplier=-1,
            pattern=[[1, 896]],
        )
        bigmask = cpool.tile([128, 896], bf16, name="bigmask")
        nc.vector.tensor_copy(bigmask, bigmask_f)

        def load_cast(dst_bf, j, runs):
            """DMA one 128-row chunk into a stage tile, cast bf16 into dst."""
            st = stpool.tile([128, W], f32, name="st", tag="st")
            for dst, (dram, src, cnt) in runs:
                nc.sync.dma_start(st[dst:dst + cnt, :],
                                  dram[src:src + cnt, :])
            nc.vector.tensor_copy(dst_bf[:, j * W:(j + 1) * W], st)

        def kv_runs(b, j, new_d, cache_d):
            if j * 128 < OFF:
                return [(dst, (cache_d, src, cnt))
                        for dst, src, cnt in _cache_runs(bt, b, j)]
            return [(0, (new_d, b * S + (j * 128 - OFF), 128))]

        # Per (b, h) transpose work is emitted lazily so it can be
        # interleaved into the previous head's matmul stream (keeps the PE
        # HAM clock-gate warm: transpose-mode doesn't count as PE-busy).
        def make_transpose_ops(k_bf, q_bf, h, tag):
            kt_sb = ktpool.tile([128, P], bf16, name=f"kt{tag}", tag="kt")
            qt_sb = qtpool.tile([128, S], bf16, name=f"qt{tag}", tag="qt")
            ops = []

            def tr(src_sb, col0, dst_sb, dcol0):
                def run():
                    tr_ps = trpool.tile([128, 128], bf16, name="tr_ps",
                                        tag="tr")
                    nc.tensor.transpose(
                        tr_ps, src_sb[:, col0:col0 + 128], ident_bf)
                    nc.vector.tensor_copy(
                        dst_sb[:, dcol0:dcol0 + 128], tr_ps)
                return run

            for j in range(NJ):
                ops.append(tr(k_bf, j * W + h * D, kt_sb, j * 128))
            for i in range(NI):
                ops.append(tr(q_bf, i * W + h * D, qt_sb, i * 128))
            return kt_sb, qt_sb, ops

        # Stage 1: load + cast all sequences' K/V/Q up front (DMA + GPSIMD
        # run ahead; SBUF holds both seqs in bf16: (16+16+8)KB * 2).
        kq = []  # per b: (k_bf, v_bf, q_bf)
        for b in range(B):
            k_bf = kpool.tile([128, NJ * W], bf16, name=f"k_bf{b}", tag="k")
            v_bf = vpool.tile([128, NJ * W], bf16, name=f"v_bf{b}", tag="v")
            q_bf = qpool.tile([128, NI * W], bf16, name=f"q_bf{b}", tag="q")
            for j in range(NJ):
                load_cast(k_bf, j, kv_runs(b, j, kn_d, kc_d))
                load_cast(v_bf, j, kv_runs(b, j, vn_d, vc_d))
            for i in range(NI):
                load_cast(q_bf, i, [(0, (q_d, b * S + i * 128, 128))])
            kq.append((k_bf, v_bf, q_bf))

        # Stage 2: per (b, h): matmul stream with next head's transposes
        # interleaved.
        heads = [(b, h) for b in range(B) for h in range(HL)]
        k_bf, v_bf, q_bf = kq[0]
        kt_sb, qt_sb, ops0 = make_transpose_ops(k_bf, q_bf, 0, "00")
        for op in ops0:
            op()
        pending = []  # transpose ops for the next head, emitted interleaved

        for hi, (b, h) in enumerate(heads):
            k_bf, v_bf, q_bf = kq[b]
            if hi + 1 < len(heads):
                nb_, nh = heads[hi + 1]
                nkt, nqt, pending = make_transpose_ops(
                    kq[nb_][0], kq[nb_][2], nh, f"{nb_}{nh}")
            else:
                nkt, nqt, pending = None, None, []

            # j-major: both s-blocks of chunk j share one PSUM tile and
            # a single wide exp; denominator matmuls are batched at the end
            # of each s-block (ones weights stay loaded).
            live = {k: [j for j in range(NJ)
                        if OFF + SBLK * k - 128 * j > -SBLK]
                    for k in range(NK)}
            ot_tiles = {k: otpool.tile([128, SBLK], f32,
                                       name=f"ot_ps{k}", tag="ot")
                        for k in range(NK)}
            rs_ps = rspool.tile([128, SBLK], f32, name="rs_ps", tag="rs")
            ex_tiles = {}
            prev = None  # (j, ks) with AV not yet emitted

            def emit_av(j, ks):
                for ki, k in enumerate(ks):
                    nc.tensor.matmul(
                        ot_tiles[k],
                        lhsT=v_bf[:, j * W + h * D:j * W + (h + 1) * D],
                        rhs=ex_tiles[j][:, ki * SBLK:(ki + 1) * SBLK],
                        start=(j == live[k][0]), stop=(j == live[k][-1]))

            for j in range(NJ):
                ks = [k for k in range(NK) if j in live[k]]
                nks = len(ks)
                qk_ps = qkpool.tile([128, NK * SBLK], f32, name="qk_ps",
                                    tag="qk")
                for ki, k in enumerate(ks):
                    nc.tensor.matmul(
                        qk_ps[:, ki * SBLK:(ki + 1) * SBLK],
                        lhsT=kt_sb[:, j * 128:(j + 1) * 128],
                        rhs=qt_sb[:, k * SBLK:(k + 1) * SBLK],
                        start=True, stop=True)
                ex = expool.tile([128, NK * SBLK], bf16, name="ex", tag="ex")
                ex_tiles[j] = ex
                nc.scalar.activation(ex[:, :nks * SBLK],
                                     qk_ps[:, :nks * SBLK], Exp, scale=SCALE)
                for ki, k in enumerate(ks):
                    base = OFF + SBLK * k - 128 * j
                    if base <= 126:  # diagonal tile: zero masked entries
                        assert 0 <= base + 384 <= 384 and base % 128 == 0
                        nc.gpsimd.tensor_mul(
                            ex[:, ki * SBLK:(ki + 1) * SBLK],
                            ex[:, ki * SBLK:(ki + 1) * SBLK],
                            bigmask[:, base + 384:base + 896])
                if prev is not None:
                    emit_av(*prev)
                prev = (j, ks)
                # drip next head's transposes into the PE stream
                if pending:
                    pending.pop(0)()
                if pending and j % 2 == 0:
                    pending.pop(0)()
            emit_av(*prev)

            # ---- denominators: batched ones-matmuls, one PSUM bank, the
            # two s-blocks packed at partition rows 0 and 32 ----
            for k in range(NK):
                for j in live[k]:
                    ki = [kk for kk in range(NK) if j in live[kk]].index(k)
                    nc.tensor.matmul(
                        rs_ps[32 * k:32 * k + 1, :], lhsT=ones[:, 0:1],
                        rhs=ex_tiles[j][:, ki * SBLK:(ki + 1) * SBLK],
                        start=(j == live[k][0]), stop=(j == live[k][-1]),
                        tile_position=(0, 32 * k))

            # ---- finalize: transpose O^T back, normalize, store ----
            for k in range(NK):
                rs_sb = finpool.tile([1, SBLK], f32, name="rs_sb",
                                     tag="rs_sb")
                nc.vector.tensor_copy(rs_sb, rs_ps[32 * k:32 * k + 1, :])
                ot_sb = finpool.tile([128, SBLK], bf16, name="ot_sb",
                                     tag="ot_sb")
                nc.scalar.copy(ot_sb, ot_tiles[k])
                for t in range(SBLK // 128):
                    rt_ps = trpool.tile([128, 1], f32, name="rt_ps",
                                        tag="tr")
                    nc.tensor.transpose(
                        rt_ps, rs_sb[0:1, t * 128:(t + 1) * 128],
                        ident[0:1, 0:1])
                    rc_sb = finpool.tile([128, 1], f32, name="rc_sb",
                                         tag="rc")
                    nc.vector.reciprocal(rc_sb, rt_ps)
                    o_ps = trpool.tile([128, 128], bf16, name="o_ps",
                                       tag="tr")
                    nc.tensor.transpose(
                        o_ps, ot_sb[:, t * 128:(t + 1) * 128], ident_bf)
                    o_sb = outpool.tile([128, 128], f32, name="o_sb",
                                        tag="o_sb")
                    nc.vector.tensor_scalar_mul(o_sb, o_ps, rc_sb)
                    row = b * S + k * SBLK + t * 128
                    nc.sync.dma_start(
                        o_d[row:row + 128, h * D:(h + 1) * D], o_sb)

            # any leftover transposes for the next head
            for op in pending:
                op()
            if nkt is not None:
                kt_sb, qt_sb = nkt, nqt

    nc.compile()
    return nc


def get_nc(block_tables):
    bt = np.asarray(block_tables)
    key = bt.tobytes()
    if key not in _CACHE:
        _CACHE[key] = _build_nc(bt)
    return _CACHE[key]


def _in_maps(query, key, value, kv_cache):
    maps = []
    for c in range(NCORES):
        cs = slice(c * W, (c + 1) * W)
        maps.append({
            "q": np.ascontiguousarray(query[:, cs]),
            "kn": np.ascontiguousarray(key[:, cs]),
            "vn": np.ascontiguousarray(value[:, cs]),
            "kc": np.ascontiguousarray(
                kv_cache[0].reshape(NB * BS, HD)[:, cs]),
            "vc": np.ascontiguousarray(
                kv_cache[1].reshape(NB * BS, HD)[:, cs]),
        })
    return maps


def run(query, key, value, kv_cache, block_tables, num_heads, **hw_kwargs):
    from concourse import bass_utils

    query = np.asarray(query, dtype=np.float32)
    key = np.asarray(key, dtype=np.float32)
    value = np.asarray(value, dtype=np.float32)
    kv_cache = np.asarray(kv_cache, dtype=np.float32)
    block_tables = np.asarray(block_tables)
    assert int(num_heads) == H
    assert query.shape == (T, HD) and kv_cache.shape == (2, NB, BS, HD)
    assert block_tables.shape == (B, BLKS)

    nc = get_nc(block_tables)
    res = bass_utils.run_bass_kernel_spmd(
        nc, _in_maps(query, key, value, kv_cache),
        core_ids=list(range(NCORES)), **hw_kwargs)
    out = np.concatenate([res.results[c]["o"] for c in range(NCORES)], axis=1)
    return out, res


def kernel(query, key, value, kv_cache, block_tables, num_heads):
    out, _ = run(query, key, value, kv_cache, block_tables, num_heads)
    return out
